# revision 7
# baseline (speedup 1.0000x reference)
"""LSHLinear Trainium2 kernel — 8-core tensor-parallel over the output dim.

Reference computation (see problem):
    in_codes = simhash(x, R)   [B,S,T]   (sign bits of x @ R^T packed per table)
    w_codes  = simhash(W, R)   [O,T]
    mask[b,s,o] = OR_t (in_codes[b,s,t] == w_codes[o,t])
    out = where(mask, x @ W^T + b, 0)

Sharding: W, b over O across 8 cores (slab of 4096 each); x, R replicated.
Each core computes its output slab TRANSPOSED (out_t[o, bs]) so that the
bias and the w-codes are per-partition quantities; host re-assembles.

Device-side per core:
  - proj matmuls (x@R^T, W@R^T) in true fp32 (4-pass) — sign decisions must
    match the fp32 reference almost exactly or the mask flips buckets
  - bits = (proj > 0) on DVE; codes packed AND broadcast across partitions
    via small bf16 matmuls with host-provided selector matrices
  - dense x@W^T in bf16 (single pass, fp32 PSUM accumulate; ~0.2% rel err
    which is far inside the tolerance)
  - bias add on the Scalar engine during PSUM eviction
  - mask built with 8 compare(-and-max) DVE/GPSIMD ops per output tile
  - masked multiply on GPSIMD, DMA out
"""

import os
import sys
import types

import numpy as np
import ml_dtypes

# ---------------------------------------------------------------------------
# Problem constants (hardcoded per the harness contract)
B, S, D, O, T, H = 4, 512, 1024, 32768, 8, 8
N_CORES = 8
BS = B * S                 # 2048 flattened tokens
OSH = O // N_CORES         # 4096 output rows per core
DT = D // 128              # 8 contraction tiles
TH = T * H                 # 64 projection rows
N_CH = BS // 512           # 4 token chunks of 512
O_CH = OSH // 512          # 8 o chunks of 512
O_TILES_PER_CH = 4         # 4 o-tiles (128 rows) per o chunk

BF16 = ml_dtypes.bfloat16

# how much mask work lands on gpsimd vs the vector engine
# (walrus rejects TensorScalarPtr on the Pool engine — keep STT on vector)
_N_GPSIMD_TABLES = int(os.environ.get("LSH_GPSIMD_TABLES", "0"))
_APPLY_ON_GPSIMD = os.environ.get("LSH_APPLY_GPSIMD", "0") == "1"


def _install_ntff_shim():
    """bass_utils wants antenv.axon_hooks for trace=True; shim it from the
    agent-boot ctypes hook when the module is missing (best effort)."""
    try:
        import antenv.axon_hooks  # noqa: F401
        return
    except ImportError:
        pass
    try:
        import antenv  # noqa: F401
        import trn_agent_boot.trn_boot as _tb
        so = "/opt/axon/libaxon_pjrt.so"
        hook = _tb._ntff_profile_via_ctypes(so) if os.path.exists(so) else None
        mod = types.ModuleType("antenv.axon_hooks")
        mod.get_axon_ntff_profile_hook = lambda: hook
        mod.set_axon_ntff_profile_hook = lambda h: None
        sys.modules["antenv.axon_hooks"] = mod
    except Exception:
        pass


_install_ntff_shim()

import concourse.bass as bass            # noqa: E402,F401
import concourse.mybir as mybir          # noqa: E402
import concourse.tile as tile            # noqa: E402
from concourse import bacc               # noqa: E402
from concourse.bass_utils import run_bass_kernel_spmd  # noqa: E402

F32 = mybir.dt.float32
BF = mybir.dt.bfloat16
ALU = mybir.AluOpType
ACTF = mybir.ActivationFunctionType


def _build_program():
    nc = bacc.Bacc("TRN2", target_bir_lowering=False, debug=False,
                   num_devices=N_CORES)

    xt_d = nc.dram_tensor("xt", [D, BS], F32, kind="ExternalInput").ap()
    xtb_d = nc.dram_tensor("xtb", [D, BS], BF, kind="ExternalInput").ap()
    wt_d = nc.dram_tensor("wt", [D, OSH], F32, kind="ExternalInput").ap()
    wtb_d = nc.dram_tensor("wtb", [D, OSH], BF, kind="ExternalInput").ap()
    rt_d = nc.dram_tensor("rt", [D, TH], F32, kind="ExternalInput").ap()
    bt_d = nc.dram_tensor("bt", [128, OSH // 128], F32, kind="ExternalInput").ap()
    btall_d = nc.dram_tensor("btall", [TH, T * 128], BF, kind="ExternalInput").ap()
    powmat_d = nc.dram_tensor("powmat", [TH, T], BF, kind="ExternalInput").ap()
    ident_d = nc.dram_tensor("ident", [128, 128], F32, kind="ExternalInput").ap()
    out_d = nc.dram_tensor("out_t", [OSH, BS], F32, kind="ExternalOutput").ap()

    with tile.TileContext(nc) as tc:
        with (
            tc.tile_pool(name="const", bufs=1) as cpool,
            tc.tile_pool(name="wt", bufs=2) as wtpool,
            tc.tile_pool(name="wstage", bufs=2) as wspool,
            tc.tile_pool(name="wcod", bufs=10) as wcpool,
            tc.tile_pool(name="work", bufs=4) as workpool,
        ):
            # ---- constants / resident tensors -----------------------------
            xtb_sb = cpool.tile([128, DT * BS], BF, tag="xtb")
            for dt in range(DT):
                nc.sync.dma_start(
                    out=xtb_sb[:, dt * BS:(dt + 1) * BS],
                    in_=xtb_d[dt * 128:(dt + 1) * 128, :],
                )
            rt_sb = cpool.tile([128, DT * TH], F32, tag="rt")
            for dt in range(DT):
                nc.sync.dma_start(
                    out=rt_sb[:, dt * TH:(dt + 1) * TH],
                    in_=rt_d[dt * 128:(dt + 1) * 128, :],
                )
            btall_sb = cpool.tile([TH, T * 128], BF, tag="btall")
            nc.sync.dma_start(out=btall_sb[:], in_=btall_d[:])
            powmat_sb = cpool.tile([TH, T], BF, tag="powmat")
            nc.sync.dma_start(out=powmat_sb[:], in_=powmat_d[:])
            ident_sb = cpool.tile([128, 128], F32, tag="ident")
            nc.sync.dma_start(out=ident_sb[:], in_=ident_d[:])
            bt_sb = cpool.tile([128, OSH // 128], F32, tag="bt")
            nc.sync.dma_start(out=bt_sb[:], in_=bt_d[:])

            # ---- stage 1: input codes, broadcast across partitions --------
            # in_proj^T [th, bs] in true fp32; bits = (proj > 0);
            # incodeB[t][p, bs] = code_t(bs) for every partition p.
            ibits = cpool.tile([TH, BS], BF, tag="ibits")
            incodeB = []
            for t in range(T):
                icb = cpool.tile([128, BS], BF, tag=f"incode{t}")
                incodeB.append(icb)
            with (
                tc.tile_pool(name="s1sb", bufs=2) as s1pool,
                tc.tile_pool(name="ps_s1", bufs=2, space="PSUM") as ps1,
            ):
                for c in range(N_CH):
                    # fp32 x^T chunk [d, 512] for all 8 d-tiles (streamed)
                    xtf = s1pool.tile([128, DT * 512], F32, tag="xtf")
                    for dt in range(DT):
                        nc.sync.dma_start(
                            out=xtf[:, dt * 512:(dt + 1) * 512],
                            in_=xt_d[dt * 128:(dt + 1) * 128,
                                     c * 512:(c + 1) * 512],
                        )
                    ip_ps = ps1.tile([TH, 512], F32, tag="ip")
                    for dt in range(DT):
                        nc.tensor.matmul(
                            ip_ps[:],
                            lhsT=rt_sb[:, dt * TH:(dt + 1) * TH],
                            rhs=xtf[:, dt * 512:(dt + 1) * 512],
                            start=(dt == 0), stop=(dt == DT - 1),
                        )
                    nc.vector.tensor_scalar(
                        out=ibits[:, c * 512:(c + 1) * 512], in0=ip_ps[:],
                        scalar1=0.0, scalar2=None, op0=ALU.is_gt,
                    )
                for t in range(T):
                    for c in range(N_CH):
                        bc_ps = ps1.tile([128, 512], F32, tag="bc")
                        nc.tensor.matmul(
                            bc_ps[:],
                            lhsT=btall_sb[:, t * 128:(t + 1) * 128],
                            rhs=ibits[:, c * 512:(c + 1) * 512],
                            start=True, stop=True,
                        )
                        nc.scalar.activation(
                            incodeB[t][:, c * 512:(c + 1) * 512], bc_ps[:],
                            ACTF.Copy,
                        )

            # ---- stage 2: per o-chunk: w-codes then dense+mask ------------
            with (
                tc.tile_pool(name="ps_w", bufs=1, space="PSUM") as psw,
                tc.tile_pool(name="ps_d", bufs=4, space="PSUM") as psd,
            ):
                for oc in range(O_CH):
                    # W^T chunk [d, 512] fp32 (for w_proj) + bf16 (for dense)
                    wt_sb = wtpool.tile([128, DT * 512], F32, tag="wt")
                    wtb_sb = wtpool.tile([128, DT * 512], BF, tag="wtb")
                    for dt in range(DT):
                        nc.sync.dma_start(
                            out=wt_sb[:, dt * 512:(dt + 1) * 512],
                            in_=wt_d[dt * 128:(dt + 1) * 128,
                                     oc * 512:(oc + 1) * 512],
                        )
                        nc.sync.dma_start(
                            out=wtb_sb[:, dt * 512:(dt + 1) * 512],
                            in_=wtb_d[dt * 128:(dt + 1) * 128,
                                      oc * 512:(oc + 1) * 512],
                        )

                    # w_proj^T [th, o512] true fp32 4-pass
                    wp_ps = psw.tile([TH, 512], F32, tag="wp")
                    for dt in range(DT):
                        nc.tensor.matmul(
                            wp_ps[:],
                            lhsT=rt_sb[:, dt * TH:(dt + 1) * TH],
                            rhs=wt_sb[:, dt * 512:(dt + 1) * 512],
                            start=(dt == 0), stop=(dt == DT - 1),
                        )
                    wbits = wspool.tile([TH, 512], BF, tag="wbits")
                    nc.vector.tensor_scalar(
                        out=wbits[:], in0=wp_ps[:], scalar1=0.0, scalar2=None,
                        op0=ALU.is_gt,
                    )
                    # pack -> w_code^T [8, 512]
                    pk_ps = psw.tile([T, 512], F32, tag="pk")
                    nc.tensor.matmul(pk_ps[:], lhsT=powmat_sb[:], rhs=wbits[:],
                                     start=True, stop=True)
                    wcodT = wspool.tile([T, 512], F32, tag="wcodT")
                    nc.scalar.activation(wcodT[:], pk_ps[:], ACTF.Copy)

                    # transpose to per-partition codes [128, 8] per o-tile
                    wcods = []
                    for j4 in range(O_TILES_PER_CH):
                        tr_ps = psw.tile([128, T], F32, tag="tr")
                        nc.tensor.transpose(
                            tr_ps[:], wcodT[:, j4 * 128:(j4 + 1) * 128],
                            ident_sb[0:T, 0:T],
                        )
                        wcod = wcpool.tile([128, T], F32, tag="wcod")
                        nc.scalar.activation(wcod[:], tr_ps[:], ACTF.Copy)
                        wcods.append(wcod)

                    # dense + mask + apply for the 4 o-tiles of this chunk
                    for j4 in range(O_TILES_PER_CH):
                        j = oc * O_TILES_PER_CH + j4
                        wcod = wcods[j4]
                        for c in range(N_CH):
                            pd = psd.tile([128, 512], F32, tag="pd")
                            for dt in range(DT):
                                nc.tensor.matmul(
                                    pd[:],
                                    lhsT=wtb_sb[:, dt * 512 + j4 * 128:
                                                dt * 512 + (j4 + 1) * 128],
                                    rhs=xtb_sb[:, dt * BS + c * 512:
                                               dt * BS + (c + 1) * 512],
                                    start=(dt == 0), stop=(dt == DT - 1),
                                )
                            dense = workpool.tile([128, 512], F32, tag="dense")
                            nc.scalar.activation(
                                dense[:], pd[:], ACTF.Identity,
                                bias=bt_sb[:, j:j + 1],
                            )
                            acc = workpool.tile([128, 512], BF, tag="acc")
                            nc.vector.tensor_scalar(
                                out=acc[:],
                                in0=incodeB[0][:, c * 512:(c + 1) * 512],
                                scalar1=wcod[:, 0:1], scalar2=None,
                                op0=ALU.is_equal,
                            )
                            for t in range(1, T):
                                eng = (nc.gpsimd if t >= T - _N_GPSIMD_TABLES
                                       else nc.vector)
                                eng.scalar_tensor_tensor(
                                    out=acc[:],
                                    in0=incodeB[t][:, c * 512:(c + 1) * 512],
                                    scalar=wcod[:, t:t + 1],
                                    in1=acc[:], op0=ALU.is_equal, op1=ALU.max,
                                )
                            res = workpool.tile([128, 512], F32, tag="res")
                            eng = nc.gpsimd if _APPLY_ON_GPSIMD else nc.vector
                            eng.tensor_tensor(out=res[:], in0=dense[:],
                                              in1=acc[:], op=ALU.mult)
                            nc.sync.dma_start(
                                out=out_d[j * 128:(j + 1) * 128,
                                          c * 512:(c + 1) * 512],
                                in_=res[:],
                            )
    nc.compile()
    return nc


_CACHE = {}


def _get_program():
    if "nc" not in _CACHE:
        _CACHE["nc"] = _build_program()
    return _CACHE["nc"]


def _host_inputs(x, W, b, R):
    """Shard + lay out inputs for the 8 cores."""
    x = np.asarray(x, np.float32)
    W = np.asarray(W, np.float32)
    b = np.asarray(b, np.float32)
    R = np.asarray(R, np.float32)

    xt = np.ascontiguousarray(x.reshape(BS, D).T)           # [D, BS] f32
    xtb = xt.astype(BF16)                                   # [D, BS] bf16
    rt = np.ascontiguousarray(R.reshape(TH, D).T)           # [D, TH]

    powers = (2.0 ** np.arange(H)).astype(np.float32)
    btall = np.zeros((TH, T * 128), np.float32)
    powmat = np.zeros((TH, T), np.float32)
    for t in range(T):
        for h in range(H):
            btall[t * H + h, t * 128:(t + 1) * 128] = powers[h]
            powmat[t * H + h, t] = powers[h]
    btall = btall.astype(BF16)
    powmat = powmat.astype(BF16)
    ident = np.eye(128, dtype=np.float32)

    wt_full = np.ascontiguousarray(W.T)                     # [D, O]
    wtb_full = wt_full.astype(BF16)

    in_maps = []
    for i in range(N_CORES):
        sl = slice(i * OSH, (i + 1) * OSH)
        wt = np.ascontiguousarray(wt_full[:, sl])           # [D, 4096] f32
        wtb = np.ascontiguousarray(wtb_full[:, sl])         # [D, 4096] bf16
        bslab = b[sl]
        bt = np.ascontiguousarray(bslab.reshape(OSH // 128, 128).T)  # [128, 32]
        in_maps.append({
            "xt": xt, "xtb": xtb, "wt": wt, "wtb": wtb, "rt": rt, "bt": bt,
            "btall": btall, "powmat": powmat, "ident": ident,
        })
    return in_maps


LAST_RESULT = None


def kernel(x, W, b, R):
    global LAST_RESULT
    nc = _get_program()
    in_maps = _host_inputs(x, W, b, R)
    res = run_bass_kernel_spmd(nc, in_maps, core_ids=list(range(N_CORES)))
    LAST_RESULT = res
    full_t = np.concatenate([res.results[i]["out_t"] for i in range(N_CORES)],
                            axis=0)                          # [O, BS]
    out = np.ascontiguousarray(full_t.T).reshape(B, S, O)
    return out.astype(np.float32, copy=False)


if __name__ == "__main__":
    rng = np.random.default_rng(0)
    x = rng.standard_normal((B, S, D), dtype=np.float32)
    W = (rng.standard_normal((O, D), dtype=np.float32) * 0.02).astype(np.float32)
    b = (rng.standard_normal((O,), dtype=np.float32) * 0.02).astype(np.float32)
    R = rng.standard_normal((T, H, D), dtype=np.float32)
    out = kernel(x, W, b, R)
    print("out", out.shape, out.dtype, float(np.abs(out).max()))


# revision 9
# speedup vs baseline: 1.2881x; 1.2881x over previous
"""LSHLinear Trainium2 kernel — 8-core tensor-parallel over the output dim.

Reference computation (see problem):
    in_codes = simhash(x, R)   [B,S,T]   (sign bits of x @ R^T packed per table)
    w_codes  = simhash(W, R)   [O,T]
    mask[b,s,o] = OR_t (in_codes[b,s,t] == w_codes[o,t])
    out = where(mask, x @ W^T + b, 0)

Sharding: W, b over O across 8 cores (slab of 4096 each); x, R replicated.
Each core computes its output slab TRANSPOSED (out_t[o, bs]) so that the
bias and the w-codes are per-partition quantities; host re-assembles.

Device-side per core:
  - proj matmuls (x@R^T, W@R^T) in true fp32 (4-pass) — sign decisions must
    match the fp32 reference almost exactly or the mask flips buckets
  - bits = (proj > 0) on DVE; codes packed AND broadcast across partitions
    via small bf16 matmuls with host-provided selector matrices
  - dense x@W^T in bf16 (single pass, fp32 PSUM accumulate; ~0.2% rel err
    which is far inside the tolerance)
  - bias add on the Scalar engine during PSUM eviction
  - mask built with 8 compare(-and-max) DVE/GPSIMD ops per output tile
  - masked multiply on GPSIMD, DMA out
"""

import os
import sys
import types

import numpy as np
import ml_dtypes

# ---------------------------------------------------------------------------
# Problem constants (hardcoded per the harness contract)
B, S, D, O, T, H = 4, 512, 1024, 32768, 8, 8
N_CORES = 8
BS = B * S                 # 2048 flattened tokens
OSH = O // N_CORES         # 4096 output rows per core
DT = D // 128              # 8 contraction tiles
TH = T * H                 # 64 projection rows
N_CH = BS // 512           # 4 token chunks of 512
O_CH = OSH // 512          # 8 o chunks of 512
O_TILES_PER_CH = 4         # 4 o-tiles (128 rows) per o chunk

BF16 = ml_dtypes.bfloat16

# how much mask work lands on gpsimd vs the vector engine
# (walrus rejects TensorScalarPtr on the Pool engine — keep STT on vector)
_N_GPSIMD_TABLES = int(os.environ.get("LSH_GPSIMD_TABLES", "0"))
_APPLY_ON_GPSIMD = os.environ.get("LSH_APPLY_GPSIMD", "0") == "1"


def _install_ntff_shim():
    """bass_utils wants antenv.axon_hooks for trace=True; shim it from the
    agent-boot ctypes hook when the module is missing (best effort)."""
    try:
        import antenv.axon_hooks  # noqa: F401
        return
    except ImportError:
        pass
    try:
        import antenv  # noqa: F401
        import trn_agent_boot.trn_boot as _tb
        so = "/opt/axon/libaxon_pjrt.so"
        hook = _tb._ntff_profile_via_ctypes(so) if os.path.exists(so) else None
        mod = types.ModuleType("antenv.axon_hooks")
        mod.get_axon_ntff_profile_hook = lambda: hook
        mod.set_axon_ntff_profile_hook = lambda h: None
        sys.modules["antenv.axon_hooks"] = mod
    except Exception:
        pass


_install_ntff_shim()

import concourse.bass as bass            # noqa: E402,F401
import concourse.mybir as mybir          # noqa: E402
import concourse.tile as tile            # noqa: E402
from concourse import bacc               # noqa: E402
from concourse.bass_utils import run_bass_kernel_spmd  # noqa: E402

F32 = mybir.dt.float32
BF = mybir.dt.bfloat16
ALU = mybir.AluOpType
ACTF = mybir.ActivationFunctionType


def _build_program():
    nc = bacc.Bacc("TRN2", target_bir_lowering=False, debug=False,
                   num_devices=N_CORES)

    xt_d = nc.dram_tensor("xt", [D, BS], F32, kind="ExternalInput").ap()
    xtb_d = nc.dram_tensor("xtb", [D, BS], BF, kind="ExternalInput").ap()
    wt_d = nc.dram_tensor("wt", [D, OSH], F32, kind="ExternalInput").ap()
    wtb_d = nc.dram_tensor("wtb", [D, OSH], BF, kind="ExternalInput").ap()
    rt_d = nc.dram_tensor("rt", [D, TH], F32, kind="ExternalInput").ap()
    bt_d = nc.dram_tensor("bt", [128, OSH // 128], F32, kind="ExternalInput").ap()
    btall_d = nc.dram_tensor("btall", [TH, T * 128], BF, kind="ExternalInput").ap()
    powmat_d = nc.dram_tensor("powmat", [TH, T], BF, kind="ExternalInput").ap()
    ident_d = nc.dram_tensor("ident", [128, 128], F32, kind="ExternalInput").ap()
    out_d = nc.dram_tensor("out_t", [OSH, BS], F32, kind="ExternalOutput").ap()

    with tile.TileContext(nc) as tc:
        with (
            tc.tile_pool(name="const", bufs=1) as cpool,
            tc.tile_pool(name="wt", bufs=2) as wtpool,
            tc.tile_pool(name="wstage", bufs=2) as wspool,
            tc.tile_pool(name="wcod", bufs=10) as wcpool,
            tc.tile_pool(name="work", bufs=4) as workpool,
        ):
            # ---- constants / resident tensors -----------------------------
            xtb_sb = cpool.tile([128, DT * BS], BF, tag="xtb")
            for dt in range(DT):
                nc.sync.dma_start(
                    out=xtb_sb[:, dt * BS:(dt + 1) * BS],
                    in_=xtb_d[dt * 128:(dt + 1) * 128, :],
                )
            rt_sb = cpool.tile([128, DT * TH], F32, tag="rt")
            for dt in range(DT):
                nc.sync.dma_start(
                    out=rt_sb[:, dt * TH:(dt + 1) * TH],
                    in_=rt_d[dt * 128:(dt + 1) * 128, :],
                )
            btall_sb = cpool.tile([TH, T * 128], BF, tag="btall")
            nc.sync.dma_start(out=btall_sb[:], in_=btall_d[:])
            powmat_sb = cpool.tile([TH, T], BF, tag="powmat")
            nc.sync.dma_start(out=powmat_sb[:], in_=powmat_d[:])
            ident_sb = cpool.tile([128, 128], F32, tag="ident")
            nc.sync.dma_start(out=ident_sb[:], in_=ident_d[:])
            bt_sb = cpool.tile([128, OSH // 128], F32, tag="bt")
            nc.sync.dma_start(out=bt_sb[:], in_=bt_d[:])
            zeros_sb = cpool.tile([128, 512], F32, tag="zeros")
            nc.gpsimd.memset(zeros_sb[:], 0.0)

            # ---- stage 1: input codes, broadcast across partitions --------
            # in_proj^T [th, bs] in true fp32; bits = (proj > 0);
            # incodeB[t][p, bs] = code_t(bs) for every partition p.
            ibits = cpool.tile([TH, BS], BF, tag="ibits")
            incodeB = []
            for t in range(T):
                icb = cpool.tile([128, BS], BF, tag=f"incode{t}")
                incodeB.append(icb)
            with (
                tc.tile_pool(name="s1sb", bufs=2) as s1pool,
                tc.tile_pool(name="ps_s1", bufs=2, space="PSUM") as ps1,
            ):
                for c in range(N_CH):
                    # fp32 x^T chunk [d, 512] for all 8 d-tiles (streamed)
                    xtf = s1pool.tile([128, DT * 512], F32, tag="xtf")
                    for dt in range(DT):
                        nc.sync.dma_start(
                            out=xtf[:, dt * 512:(dt + 1) * 512],
                            in_=xt_d[dt * 128:(dt + 1) * 128,
                                     c * 512:(c + 1) * 512],
                        )
                    ip_ps = ps1.tile([TH, 512], F32, tag="ip")
                    for dt in range(DT):
                        nc.tensor.matmul(
                            ip_ps[:],
                            lhsT=rt_sb[:, dt * TH:(dt + 1) * TH],
                            rhs=xtf[:, dt * 512:(dt + 1) * 512],
                            start=(dt == 0), stop=(dt == DT - 1),
                        )
                    nc.vector.tensor_scalar(
                        out=ibits[:, c * 512:(c + 1) * 512], in0=ip_ps[:],
                        scalar1=0.0, scalar2=None, op0=ALU.is_gt,
                    )
                for t in range(T):
                    for c in range(N_CH):
                        bc_ps = ps1.tile([128, 512], F32, tag="bc")
                        nc.tensor.matmul(
                            bc_ps[:],
                            lhsT=btall_sb[:, t * 128:(t + 1) * 128],
                            rhs=ibits[:, c * 512:(c + 1) * 512],
                            start=True, stop=True,
                        )
                        nc.scalar.activation(
                            incodeB[t][:, c * 512:(c + 1) * 512], bc_ps[:],
                            ACTF.Copy,
                        )

            # ---- stage 2: per o-chunk: w-codes then dense+mask ------------
            with (
                tc.tile_pool(name="ps_w", bufs=1, space="PSUM") as psw,
                tc.tile_pool(name="ps_d", bufs=4, space="PSUM") as psd,
            ):
                for oc in range(O_CH):
                    # W^T chunk [d, 512] fp32 (for w_proj) + bf16 (for dense)
                    wt_sb = wtpool.tile([128, DT * 512], F32, tag="wt")
                    wtb_sb = wtpool.tile([128, DT * 512], BF, tag="wtb")
                    for dt in range(DT):
                        nc.sync.dma_start(
                            out=wt_sb[:, dt * 512:(dt + 1) * 512],
                            in_=wt_d[dt * 128:(dt + 1) * 128,
                                     oc * 512:(oc + 1) * 512],
                        )
                        nc.sync.dma_start(
                            out=wtb_sb[:, dt * 512:(dt + 1) * 512],
                            in_=wtb_d[dt * 128:(dt + 1) * 128,
                                      oc * 512:(oc + 1) * 512],
                        )

                    # w_proj^T [th, o512] true fp32 4-pass
                    wp_ps = psw.tile([TH, 512], F32, tag="wp")
                    for dt in range(DT):
                        nc.tensor.matmul(
                            wp_ps[:],
                            lhsT=rt_sb[:, dt * TH:(dt + 1) * TH],
                            rhs=wt_sb[:, dt * 512:(dt + 1) * 512],
                            start=(dt == 0), stop=(dt == DT - 1),
                        )
                    wbits = wspool.tile([TH, 512], BF, tag="wbits")
                    nc.vector.tensor_scalar(
                        out=wbits[:], in0=wp_ps[:], scalar1=0.0, scalar2=None,
                        op0=ALU.is_gt,
                    )
                    # pack -> w_code^T [8, 512]
                    pk_ps = psw.tile([T, 512], F32, tag="pk")
                    nc.tensor.matmul(pk_ps[:], lhsT=powmat_sb[:], rhs=wbits[:],
                                     start=True, stop=True)
                    wcodT = wspool.tile([T, 512], F32, tag="wcodT")
                    nc.scalar.activation(wcodT[:], pk_ps[:], ACTF.Copy)

                    # transpose to per-partition codes [128, 8] per o-tile
                    wcods = []
                    for j4 in range(O_TILES_PER_CH):
                        tr_ps = psw.tile([128, T], F32, tag="tr")
                        nc.tensor.transpose(
                            tr_ps[:], wcodT[:, j4 * 128:(j4 + 1) * 128],
                            ident_sb[0:T, 0:T],
                        )
                        wcod = wcpool.tile([128, T], F32, tag="wcod")
                        nc.scalar.activation(wcod[:], tr_ps[:], ACTF.Copy)
                        wcods.append(wcod)

                    # mask + dense + apply for the 4 o-tiles of this chunk.
                    # nm[p, bs] = prod_t (in_code_t != w_code_t): nonzero
                    # iff NO table matches -> zero those outputs.
                    for j4 in range(O_TILES_PER_CH):
                        j = oc * O_TILES_PER_CH + j4
                        wcod = wcods[j4]
                        nm = workpool.tile([128, BS], BF, tag="nm")
                        ne = workpool.tile([128, BS], BF, tag="ne")
                        nc.vector.tensor_scalar(
                            out=nm[:], in0=incodeB[0][:],
                            scalar1=wcod[:, 0:1], scalar2=None,
                            op0=ALU.not_equal,
                        )
                        for t in range(1, T):
                            nc.vector.tensor_scalar(
                                out=ne[:], in0=incodeB[t][:],
                                scalar1=wcod[:, t:t + 1], scalar2=None,
                                op0=ALU.not_equal,
                            )
                            nc.vector.tensor_tensor(out=nm[:], in0=nm[:],
                                                    in1=ne[:], op=ALU.mult)
                        for c in range(N_CH):
                            pd = psd.tile([128, 512], F32, tag="pd")
                            for dt in range(DT):
                                nc.tensor.matmul(
                                    pd[:],
                                    lhsT=wtb_sb[:, dt * 512 + j4 * 128:
                                                dt * 512 + (j4 + 1) * 128],
                                    rhs=xtb_sb[:, dt * BS + c * 512:
                                               dt * BS + (c + 1) * 512],
                                    start=(dt == 0), stop=(dt == DT - 1),
                                )
                            dense = workpool.tile([128, 512], F32, tag="dense")
                            nc.scalar.activation(
                                dense[:], pd[:], ACTF.Identity,
                                bias=bt_sb[:, j:j + 1],
                            )
                            nc.vector.copy_predicated(
                                out=dense[:],
                                mask=nm.bitcast(mybir.dt.uint16)[
                                    :, c * 512:(c + 1) * 512],
                                data=zeros_sb[:],
                            )
                            nc.sync.dma_start(
                                out=out_d[j * 128:(j + 1) * 128,
                                          c * 512:(c + 1) * 512],
                                in_=dense[:],
                            )
    nc.compile()
    return nc


_CACHE = {}


def _get_program():
    if "nc" not in _CACHE:
        _CACHE["nc"] = _build_program()
    return _CACHE["nc"]


def _host_inputs(x, W, b, R):
    """Shard + lay out inputs for the 8 cores."""
    x = np.asarray(x, np.float32)
    W = np.asarray(W, np.float32)
    b = np.asarray(b, np.float32)
    R = np.asarray(R, np.float32)

    xt = np.ascontiguousarray(x.reshape(BS, D).T)           # [D, BS] f32
    xtb = xt.astype(BF16)                                   # [D, BS] bf16
    rt = np.ascontiguousarray(R.reshape(TH, D).T)           # [D, TH]

    powers = (2.0 ** np.arange(H)).astype(np.float32)
    btall = np.zeros((TH, T * 128), np.float32)
    powmat = np.zeros((TH, T), np.float32)
    for t in range(T):
        for h in range(H):
            btall[t * H + h, t * 128:(t + 1) * 128] = powers[h]
            powmat[t * H + h, t] = powers[h]
    btall = btall.astype(BF16)
    powmat = powmat.astype(BF16)
    ident = np.eye(128, dtype=np.float32)

    wt_full = np.ascontiguousarray(W.T)                     # [D, O]
    wtb_full = wt_full.astype(BF16)

    in_maps = []
    for i in range(N_CORES):
        sl = slice(i * OSH, (i + 1) * OSH)
        wt = np.ascontiguousarray(wt_full[:, sl])           # [D, 4096] f32
        wtb = np.ascontiguousarray(wtb_full[:, sl])         # [D, 4096] bf16
        bslab = b[sl]
        bt = np.ascontiguousarray(bslab.reshape(OSH // 128, 128).T)  # [128, 32]
        in_maps.append({
            "xt": xt, "xtb": xtb, "wt": wt, "wtb": wtb, "rt": rt, "bt": bt,
            "btall": btall, "powmat": powmat, "ident": ident,
        })
    return in_maps


LAST_RESULT = None


def kernel(x, W, b, R):
    global LAST_RESULT
    nc = _get_program()
    in_maps = _host_inputs(x, W, b, R)
    res = run_bass_kernel_spmd(nc, in_maps, core_ids=list(range(N_CORES)))
    LAST_RESULT = res
    full_t = np.concatenate([res.results[i]["out_t"] for i in range(N_CORES)],
                            axis=0)                          # [O, BS]
    out = np.ascontiguousarray(full_t.T).reshape(B, S, O)
    return out.astype(np.float32, copy=False)


if __name__ == "__main__":
    rng = np.random.default_rng(0)
    x = rng.standard_normal((B, S, D), dtype=np.float32)
    W = (rng.standard_normal((O, D), dtype=np.float32) * 0.02).astype(np.float32)
    b = (rng.standard_normal((O,), dtype=np.float32) * 0.02).astype(np.float32)
    R = rng.standard_normal((T, H, D), dtype=np.float32)
    out = kernel(x, W, b, R)
    print("out", out.shape, out.dtype, float(np.abs(out).max()))
